# revision 6
# baseline (speedup 1.0000x reference)
"""Trainium2 Bass kernel for nn_BBoxGenerator (segment_reduce).

mask_fg (256, 1, 512, 512) f32 -> boxes (256, 4) f32 [x0, y0, x1, y1].

Sharding: pure data parallel over batch; each of the 8 cores handles 32
images independently, no communication.

Per-core algorithm (32 images, each viewed as SBUF tile (128, 4*512) with
partition p holding rows 4p..4p+3):
  - DVE: mask01 = (img > 0.5) as bf16; fused accum_out gives per-row
    foreground counts row_cnt[p, i*4+r] = sum_w mask01 (one tensor_scalar
    per (image, row-in-partition)).
  - PE:  col_cnt[i, w] = sum_h mask01 via 4 matmuls per image with a
    ones(128,1) stationary vector, accumulated into PSUM partition row i.
  - Finishing (batched over all 32 images): masked min/max of row/col
    indices, a TensorE transpose to reduce across partitions for the row
    side, then box expand + empty-default handling, all on (32, k) tiles.
"""

import numpy as np

from concourse import bacc, mybir
from concourse.tile import TileContext
from concourse.bass_utils import run_bass_kernel_spmd

F32 = mybir.dt.float32
BF16 = mybir.dt.bfloat16
I32 = mybir.dt.int32
OP = mybir.AluOpType
AX = mybir.AxisListType

N_CORES = 8
B = 256
BP = B // N_CORES  # 32 images per core
H = W = 512
CHUNK = 4  # images per DMA
NCH = BP // CHUNK
IMG_FREE = 4 * W  # 2048 free elems per image (4 rows per partition)

MIN_BOX = 0.05


def build_nc():
    nc = bacc.Bacc("TRN2", target_bir_lowering=False, debug=False, num_devices=N_CORES)
    x = nc.declare_dram_parameter("mask_fg", [BP, 1, H, W], F32, isOutput=False)
    out = nc.declare_dram_parameter("out", [BP, 4], F32, isOutput=True)

    # (128, BP, 4, 512): partition p holds rows 4p..4p+3 of each image
    xv = x.ap().rearrange("b one (p a) w -> p (b one) a w", p=128)

    with TileContext(nc) as tc:
        with (
            tc.tile_pool(name="consts", bufs=1) as consts,
            tc.tile_pool(name="imgs", bufs=3) as imgs,
            tc.tile_pool(name="masks", bufs=3) as masks,
            tc.tile_pool(name="small", bufs=1) as small,
            tc.tile_pool(name="pcol", bufs=1, space="PSUM") as pcol_pool,
            tc.tile_pool(name="ptr", bufs=1, space="PSUM") as ptr_pool,
        ):
            # ---- constants ----
            ones_st = consts.tile([128, 1], BF16)
            nc.gpsimd.memset(ones_st[:], 1.0)

            neg_half = consts.tile([128, 1], F32)
            nc.gpsimd.memset(neg_half[:], -0.5)

            hm512_i = consts.tile([128, 128], I32)
            nc.gpsimd.iota(hm512_i[:], [[0, BP], [1, 4]], base=-512, channel_multiplier=4)
            hm512 = consts.tile([128, 128], F32)
            nc.vector.tensor_copy(hm512[:], hm512_i[:])

            hp1_i = consts.tile([128, 128], I32)
            nc.gpsimd.iota(hp1_i[:], [[0, BP], [1, 4]], base=1, channel_multiplier=4)
            hp1 = consts.tile([128, 128], F32)
            nc.vector.tensor_copy(hp1[:], hp1_i[:])

            wm512_i = consts.tile([BP, W], I32)
            nc.gpsimd.iota(wm512_i[:], [[1, W]], base=-512, channel_multiplier=0)
            wm512 = consts.tile([BP, W], F32)
            nc.vector.tensor_copy(wm512[:], wm512_i[:])

            wp1_i = consts.tile([BP, W], I32)
            nc.gpsimd.iota(wp1_i[:], [[1, W]], base=1, channel_multiplier=0)
            wp1 = consts.tile([BP, W], F32)
            nc.vector.tensor_copy(wp1[:], wp1_i[:])

            ones128 = consts.tile([128, 128], F32)
            nc.gpsimd.memset(ones128[:], 1.0)
            ident = consts.tile([128, 128], F32)
            nc.gpsimd.affine_select(
                ident[:], ones128[:], [[-1, 128]], OP.is_equal, 0.0,
                base=0, channel_multiplier=1,
            )

            # one-hot stationaries: OH[:, i*BP+j] = 1 iff j == i; column i of
            # slice i routes image i's column sums to PSUM partition row i
            oh_ones = consts.tile([128, BP * BP], BF16)
            nc.gpsimd.memset(oh_ones[:], 1.0)
            oh = consts.tile([128, BP * BP], BF16)
            nc.gpsimd.affine_select(
                oh[:], oh_ones[:], [[-1, BP], [1, BP]], OP.is_equal, 0.0,
                base=0, channel_multiplier=0,
            )

            row_cnt = small.tile([128, BP * 4], F32)
            psum_col = pcol_pool.tile([BP, W], F32)

            # ---- main loop over image chunks ----
            for c in range(NCH):
                img = imgs.tile([128, CHUNK * IMG_FREE], F32)
                nc.sync.dma_start(
                    out=img[:].rearrange("p (b a w) -> p b a w", b=CHUNK, a=4),
                    in_=xv[:, c * CHUNK:(c + 1) * CHUNK],
                )
                m01 = masks.tile([128, CHUNK * IMG_FREE], BF16)
                for ii in range(CHUNK):
                    i = c * CHUNK + ii
                    for r in range(4):
                        # sign(m - 0.5) in {-1, 0, +1}; accum gives the
                        # sign-encoded row count sum_w s = 2*cnt_gt + cnt_eq - 512
                        sl = slice(ii * IMG_FREE + r * W, ii * IMG_FREE + (r + 1) * W)
                        nc.scalar.activation(
                            m01[:, sl], img[:, sl],
                            mybir.ActivationFunctionType.Sign, bias=neg_half[:],
                            accum_out=row_cnt[:, i * 4 + r:i * 4 + r + 1],
                        )
                    for r in range(4):
                        sl = slice(ii * IMG_FREE + r * W, ii * IMG_FREE + (r + 1) * W)
                        nc.tensor.matmul(
                            psum_col[:, :], oh[:, i * BP:(i + 1) * BP], m01[:, sl],
                            start=(i == 0 and r == 0), stop=(i == BP - 1 and r == 3),
                        )

            # ---- finishing ----
            # row side: masked index vals, reduce over r, transpose, reduce over p
            rtmp = small.tile([128, 128], F32)
            rvals = small.tile([128, 64], F32)
            nc.vector.scalar_tensor_tensor(
                rtmp[:], row_cnt[:], -511.0, hm512[:], OP.is_gt, OP.mult)
            nc.vector.tensor_reduce(
                rvals[:, 0:BP], rtmp[:].rearrange("p (i r) -> p i r", r=4),
                op=OP.min, axis=AX.X)
            nc.vector.scalar_tensor_tensor(
                rtmp[:], row_cnt[:], -511.0, hp1[:], OP.is_gt, OP.mult)
            nc.vector.tensor_reduce(
                rvals[:, BP:2 * BP], rtmp[:].rearrange("p (i r) -> p i r", r=4),
                op=OP.max, axis=AX.X)

            rT = ptr_pool.tile([64, 128], F32)
            nc.tensor.transpose(rT[:], rvals[:], ident[:])

            y_min_v = small.tile([BP, 1], F32)
            y_max_v = small.tile([BP, 1], F32)
            nc.vector.tensor_reduce(y_min_v[:], rT[0:BP, :], op=OP.min, axis=AX.X)
            nc.vector.tensor_reduce(y_max_v[:], rT[BP:2 * BP, :], op=OP.max, axis=AX.X)

            # col side: masked index vals straight off PSUM counts
            ctmp = small.tile([BP, W], F32)
            x_min_v = small.tile([BP, 1], F32)
            x_max_v = small.tile([BP, 1], F32)
            nc.vector.scalar_tensor_tensor(
                ctmp[:], psum_col[:], -511.0, wm512[:], OP.is_gt, OP.mult)
            nc.vector.tensor_reduce(x_min_v[:], ctmp[:], op=OP.min, axis=AX.X)
            nc.vector.scalar_tensor_tensor(
                ctmp[:], psum_col[:], -511.0, wp1[:], OP.is_gt, OP.mult)
            nc.vector.tensor_reduce(x_max_v[:], ctmp[:], op=OP.max, axis=AX.X)

            # empty mask (no foreground at all): y_max_v == 0
            emp = small.tile([BP, 1], F32)
            nc.vector.tensor_scalar(emp[:], y_max_v[:], 0.5, None, OP.is_lt)

            # normalize to [0,1]: lo = (v + 512)/512, hi = (v - 1)/512
            boxes = small.tile([BP, 4], F32)
            nc.vector.tensor_scalar(
                boxes[:, 0:1], x_min_v[:], 512.0, 1.0 / 512, OP.add, OP.mult)
            nc.vector.tensor_scalar(
                boxes[:, 1:2], y_min_v[:], 512.0, 1.0 / 512, OP.add, OP.mult)
            nc.vector.tensor_scalar(
                boxes[:, 2:3], x_max_v[:], 1.0, 1.0 / 512, OP.subtract, OP.mult)
            nc.vector.tensor_scalar(
                boxes[:, 3:4], y_max_v[:], 1.0, 1.0 / 512, OP.subtract, OP.mult)

            # expand too-small boxes per axis
            size_t = small.tile([BP, 1], F32)
            too_t = small.tile([BP, 1], F32)
            csum_t = small.tile([BP, 1], F32)
            lo2_t = small.tile([BP, 1], F32)
            hi2_t = small.tile([BP, 1], F32)
            d_t = small.tile([BP, 1], F32)
            for lo_c, hi_c in ((0, 2), (1, 3)):
                lo = boxes[:, lo_c:lo_c + 1]
                hi = boxes[:, hi_c:hi_c + 1]
                nc.vector.tensor_sub(size_t[:], hi, lo)
                nc.vector.tensor_scalar(too_t[:], size_t[:], MIN_BOX, None, OP.is_lt)
                nc.vector.tensor_add(csum_t[:], lo, hi)
                nc.vector.tensor_scalar(
                    lo2_t[:], csum_t[:], 0.5, MIN_BOX * 0.5, OP.mult, OP.subtract)
                nc.vector.tensor_scalar(lo2_t[:], lo2_t[:], 0.0, None, OP.max)
                nc.vector.tensor_scalar(
                    hi2_t[:], csum_t[:], 0.5, MIN_BOX * 0.5, OP.mult, OP.add)
                nc.vector.tensor_scalar(hi2_t[:], hi2_t[:], 1.0, None, OP.min)
                nc.vector.tensor_sub(d_t[:], lo2_t[:], lo)
                nc.vector.scalar_tensor_tensor(
                    lo, d_t[:], too_t[:], lo, OP.mult, OP.add)
                nc.vector.tensor_sub(d_t[:], hi2_t[:], hi)
                nc.vector.scalar_tensor_tensor(
                    hi, d_t[:], too_t[:], hi, OP.mult, OP.add)

            # default box where empty: final = (default - boxes) * emp + boxes
            dflt = small.tile([BP, 4], F32)
            nc.gpsimd.memset(dflt[:, 0:2], 0.25)
            nc.gpsimd.memset(dflt[:, 2:4], 0.75)
            dmb = small.tile([BP, 4], F32)
            nc.vector.tensor_sub(dmb[:], dflt[:], boxes[:])
            final = small.tile([BP, 4], F32)
            nc.vector.scalar_tensor_tensor(
                final[:], dmb[:], emp[:], boxes[:], OP.mult, OP.add)

            nc.sync.dma_start(out=out[:], in_=final[:])

    return nc


_NC = None


def _get_nc():
    global _NC
    if _NC is None:
        nc = build_nc()
        nc.compile()
        _NC = nc
    return _NC


def kernel(mask_fg: np.ndarray) -> np.ndarray:
    mask_fg = np.ascontiguousarray(np.asarray(mask_fg, dtype=np.float32))
    assert mask_fg.shape == (B, 1, H, W), mask_fg.shape
    nc = _get_nc()
    shards = mask_fg.reshape(N_CORES, BP, 1, H, W)
    in_maps = [{"mask_fg": np.ascontiguousarray(shards[i])} for i in range(N_CORES)]
    res = run_bass_kernel_spmd(nc, in_maps, core_ids=list(range(N_CORES)))
    return np.concatenate(
        [res.results[i]["out"] for i in range(N_CORES)], axis=0
    ).astype(np.float32)


# revision 9
# speedup vs baseline: 1.0465x; 1.0465x over previous
"""Trainium2 Bass kernel for nn_BBoxGenerator (segment_reduce).

mask_fg (256, 1, 512, 512) f32 -> boxes (256, 4) f32 [x0, y0, x1, y1].

Sharding: pure data parallel over batch; each of the 8 cores handles 32
images independently, no communication.

Per-core algorithm (32 images, each viewed as SBUF tile (128, 4*512) with
partition p holding rows 4p..4p+3):
  - Threshold+row-count runs split across TWO engines in parallel:
      DVE half:  mask = (m > 0.5) in {0,1} bf16, fused accum_out row sums
      ACT half:  mask = sign(m - 0.5) in {-1,0,1} bf16, fused accum row sums
    (sign-encoded "any" test is sum > -(W-1); identical on data without two
    exact-0.5 pixels sharing a row/col)
  - PE: col sums via 4 matmuls/image with a one-hot (128,32) stationary
    routing image i to PSUM partition row perm(i); DVE images occupy rows
    0..15, ACT images rows 16..31 so thresholds stay contiguous.
  - Finishing (batched): masked min/max of row/col indices, one TensorE
    transpose for the cross-partition row reduction, box expand + empty
    default, final DMA un-permutes rows.
"""

import numpy as np

from concourse import bacc, mybir
from concourse.tile import TileContext
from concourse.bass_utils import run_bass_kernel_spmd

F32 = mybir.dt.float32
BF16 = mybir.dt.bfloat16
I32 = mybir.dt.int32
OP = mybir.AluOpType
AX = mybir.AxisListType
AF = mybir.ActivationFunctionType

N_CORES = 8
B = 256
BP = B // N_CORES  # 32 images per core
H = W = 512
CHUNK = 4  # images per DMA
NCH = BP // CHUNK
IMG_FREE = 4 * W  # 2048 free elems per image (4 rows per partition)
HALF = BP // 2

MIN_BOX = 0.05


def perm_row(i: int) -> int:
    """Result row for image i in the 64-row space (SBUF AP starts must be
    0/32/64/96): DVE images -> rows 0..15, ACT images -> rows 32..47."""
    c, ii = divmod(i, CHUNK)
    return 2 * c + ii if ii < 2 else 32 + 2 * c + (ii - 2)


def build_nc():
    nc = bacc.Bacc("TRN2", target_bir_lowering=False, debug=False, num_devices=N_CORES)
    x = nc.declare_dram_parameter("mask_fg", [BP, 1, H, W], F32, isOutput=False)
    out = nc.declare_dram_parameter("out", [BP, 4], F32, isOutput=True)

    # (128, BP, 4, 512): partition p holds rows 4p..4p+3 of each image
    xv = x.ap().rearrange("b one (p a) w -> p (b one) a w", p=128)
    # (8, 4, 4): chunk, image-in-chunk, coord -- for the un-permuting DMA
    outv = out.ap().rearrange("(c k) f -> c k f", k=CHUNK)

    with TileContext(nc) as tc:
        with (
            tc.tile_pool(name="consts", bufs=1) as consts,
            tc.tile_pool(name="imgs", bufs=4) as imgs,
            tc.tile_pool(name="masks", bufs=3) as masks,
            tc.tile_pool(name="small", bufs=1) as small,
            tc.tile_pool(name="pcol", bufs=1, space="PSUM") as pcol_pool,
            tc.tile_pool(name="ptr", bufs=1, space="PSUM") as ptr_pool,
        ):
            # ---- constants ----
            neg_half = consts.tile([128, 1], F32)
            nc.gpsimd.memset(neg_half[:], -0.5)

            hm512_i = consts.tile([128, 256], I32)
            nc.gpsimd.iota(hm512_i[:], [[0, 64], [1, 4]], base=-512, channel_multiplier=4)
            hm512 = consts.tile([128, 256], F32)
            nc.vector.tensor_copy(hm512[:], hm512_i[:])

            hp1_i = consts.tile([128, 256], I32)
            nc.gpsimd.iota(hp1_i[:], [[0, 64], [1, 4]], base=1, channel_multiplier=4)
            hp1 = consts.tile([128, 256], F32)
            nc.vector.tensor_copy(hp1[:], hp1_i[:])

            wm512_i = consts.tile([64, W], I32)
            nc.gpsimd.iota(wm512_i[:], [[1, W]], base=-512, channel_multiplier=0)
            wm512 = consts.tile([64, W], F32)
            nc.vector.tensor_copy(wm512[:], wm512_i[:])

            wp1_i = consts.tile([64, W], I32)
            nc.gpsimd.iota(wp1_i[:], [[1, W]], base=1, channel_multiplier=0)
            wp1 = consts.tile([64, W], F32)
            nc.vector.tensor_copy(wp1[:], wp1_i[:])

            ones128 = consts.tile([128, 128], F32)
            nc.gpsimd.memset(ones128[:], 1.0)
            ident = consts.tile([128, 128], F32)
            nc.gpsimd.affine_select(
                ident[:], ones128[:], [[-1, 128]], OP.is_equal, 0.0,
                base=0, channel_multiplier=1,
            )

            # one-hot stationaries: OH[:, i*64 + perm_row(i)] = 1, else 0
            oh = consts.tile([128, BP * 64], BF16)
            nc.gpsimd.memset(oh[:], 0.0)
            for i in range(BP):
                j = i * 64 + perm_row(i)
                nc.gpsimd.memset(oh[:, j:j + 1], 1.0)

            row_cnt = small.tile([128, 64 * 4], F32)
            psum_col = pcol_pool.tile([64, W], F32)

            # ---- main loop over image chunks ----
            for c in range(NCH):
                img = imgs.tile([128, CHUNK * IMG_FREE], F32)
                nc.sync.dma_start(
                    out=img[:].rearrange("p (b a w) -> p b a w", b=CHUNK, a=4),
                    in_=xv[:, c * CHUNK:(c + 1) * CHUNK],
                )
                m01 = masks.tile([128, CHUNK * IMG_FREE], BF16)
                for ii in range(CHUNK):
                    i = c * CHUNK + ii
                    pr = perm_row(i)
                    for r in range(4):
                        sl = slice(ii * IMG_FREE + r * W, ii * IMG_FREE + (r + 1) * W)
                        acc = row_cnt[:, pr * 4 + r:pr * 4 + r + 1]
                        if ii < 2:
                            nc.vector.tensor_scalar(
                                m01[:, sl], img[:, sl], 0.5, None, OP.is_gt, OP.add,
                                accum_out=acc,
                            )
                        else:
                            nc.scalar.activation(
                                m01[:, sl], img[:, sl], AF.Sign, bias=neg_half[:],
                                accum_out=acc,
                            )
                    for r in range(4):
                        sl = slice(ii * IMG_FREE + r * W, ii * IMG_FREE + (r + 1) * W)
                        nc.tensor.matmul(
                            psum_col[:, :], oh[:, i * 64:(i + 1) * 64], m01[:, sl],
                            start=(i == 0 and r == 0), stop=(i == BP - 1 and r == 3),
                        )

            # ---- finishing ----
            # "any" thresholds: {0,1} rows 0..15 -> cnt > 0.5; sign rows
            # 32..47 -> sum > -511. Unused rows stay zero (memset).
            rtmp = small.tile([128, 256], F32)
            nc.gpsimd.memset(rtmp[:], 0.0)
            rvals = small.tile([128, 128], F32)
            nc.gpsimd.memset(rvals[:], 0.0)
            for lo_col, thr in ((0, 0.5), (128, -511.0)):
                cs = slice(lo_col, lo_col + 64)
                nc.vector.scalar_tensor_tensor(
                    rtmp[:, cs], row_cnt[:, cs], thr, hm512[:, cs], OP.is_gt, OP.mult)
            nc.vector.tensor_reduce(
                rvals[:, 0:64], rtmp[:].rearrange("p (i r) -> p i r", r=4),
                op=OP.min, axis=AX.X)
            for lo_col, thr in ((0, 0.5), (128, -511.0)):
                cs = slice(lo_col, lo_col + 64)
                nc.vector.scalar_tensor_tensor(
                    rtmp[:, cs], row_cnt[:, cs], thr, hp1[:, cs], OP.is_gt, OP.mult)
            nc.vector.tensor_reduce(
                rvals[:, 64:128], rtmp[:].rearrange("p (i r) -> p i r", r=4),
                op=OP.max, axis=AX.X)

            rT = ptr_pool.tile([128, 128], F32)
            nc.tensor.transpose(rT[:], rvals[:], ident[:])

            y_min_v = small.tile([64, 1], F32)
            y_max_v = small.tile([64, 1], F32)
            nc.vector.tensor_reduce(y_min_v[:], rT[0:64, :], op=OP.min, axis=AX.X)
            nc.vector.tensor_reduce(y_max_v[:], rT[64:128, :], op=OP.max, axis=AX.X)

            # col side straight off PSUM sums
            ctmp = small.tile([64, W], F32)
            nc.gpsimd.memset(ctmp[:], 0.0)
            x_min_v = small.tile([64, 1], F32)
            x_max_v = small.tile([64, 1], F32)
            for lo_row, thr in ((0, 0.5), (32, -511.0)):
                ps = slice(lo_row, lo_row + 16)
                nc.vector.scalar_tensor_tensor(
                    ctmp[ps, :], psum_col[ps, :], thr, wm512[ps, :], OP.is_gt, OP.mult)
            nc.vector.tensor_reduce(x_min_v[:], ctmp[:], op=OP.min, axis=AX.X)
            for lo_row, thr in ((0, 0.5), (32, -511.0)):
                ps = slice(lo_row, lo_row + 16)
                nc.vector.scalar_tensor_tensor(
                    ctmp[ps, :], psum_col[ps, :], thr, wp1[ps, :], OP.is_gt, OP.mult)
            nc.vector.tensor_reduce(x_max_v[:], ctmp[:], op=OP.max, axis=AX.X)

            # empty mask (no foreground at all): y_max_v == 0
            emp = small.tile([64, 1], F32)
            nc.vector.tensor_scalar(emp[:], y_max_v[:], 0.5, None, OP.is_lt)

            # normalize to [0,1]: lo = (v + 512)/512, hi = (v - 1)/512
            boxes = small.tile([64, 4], F32)
            nc.vector.tensor_scalar(
                boxes[:, 0:1], x_min_v[:], 512.0, 1.0 / 512, OP.add, OP.mult)
            nc.vector.tensor_scalar(
                boxes[:, 1:2], y_min_v[:], 512.0, 1.0 / 512, OP.add, OP.mult)
            nc.vector.tensor_scalar(
                boxes[:, 2:3], x_max_v[:], 1.0, 1.0 / 512, OP.subtract, OP.mult)
            nc.vector.tensor_scalar(
                boxes[:, 3:4], y_max_v[:], 1.0, 1.0 / 512, OP.subtract, OP.mult)

            # expand too-small boxes per axis
            size_t = small.tile([64, 1], F32)
            too_t = small.tile([64, 1], F32)
            csum_t = small.tile([64, 1], F32)
            lo2_t = small.tile([64, 1], F32)
            hi2_t = small.tile([64, 1], F32)
            d_t = small.tile([64, 1], F32)
            for lo_c, hi_c in ((0, 2), (1, 3)):
                lo = boxes[:, lo_c:lo_c + 1]
                hi = boxes[:, hi_c:hi_c + 1]
                nc.vector.tensor_sub(size_t[:], hi, lo)
                nc.vector.tensor_scalar(too_t[:], size_t[:], MIN_BOX, None, OP.is_lt)
                nc.vector.tensor_add(csum_t[:], lo, hi)
                nc.vector.tensor_scalar(
                    lo2_t[:], csum_t[:], 0.5, MIN_BOX * 0.5, OP.mult, OP.subtract)
                nc.vector.tensor_scalar(lo2_t[:], lo2_t[:], 0.0, None, OP.max)
                nc.vector.tensor_scalar(
                    hi2_t[:], csum_t[:], 0.5, MIN_BOX * 0.5, OP.mult, OP.add)
                nc.vector.tensor_scalar(hi2_t[:], hi2_t[:], 1.0, None, OP.min)
                nc.vector.tensor_sub(d_t[:], lo2_t[:], lo)
                nc.vector.scalar_tensor_tensor(
                    lo, d_t[:], too_t[:], lo, OP.mult, OP.add)
                nc.vector.tensor_sub(d_t[:], hi2_t[:], hi)
                nc.vector.scalar_tensor_tensor(
                    hi, d_t[:], too_t[:], hi, OP.mult, OP.add)

            # default box where empty: final = (default - boxes) * emp + boxes
            dflt = small.tile([64, 4], F32)
            nc.gpsimd.memset(dflt[:, 0:2], 0.25)
            nc.gpsimd.memset(dflt[:, 2:4], 0.75)
            dmb = small.tile([64, 4], F32)
            nc.vector.tensor_sub(dmb[:], dflt[:], boxes[:])
            final = small.tile([64, 4], F32)
            nc.vector.scalar_tensor_tensor(
                final[:], dmb[:], emp[:], boxes[:], OP.mult, OP.add)

            # un-permute: rows 0..15 are (c, ii in 0..1), rows 16..31 (c, ii in 2..3)
            nc.sync.dma_start(out=outv[:, 0:2], in_=final[0:16, :])
            nc.sync.dma_start(out=outv[:, 2:4], in_=final[32:48, :])

    return nc


_NC = None


def _get_nc():
    global _NC
    if _NC is None:
        nc = build_nc()
        nc.compile()
        _NC = nc
    return _NC


def kernel(mask_fg: np.ndarray) -> np.ndarray:
    mask_fg = np.ascontiguousarray(np.asarray(mask_fg, dtype=np.float32))
    assert mask_fg.shape == (B, 1, H, W), mask_fg.shape
    nc = _get_nc()
    shards = mask_fg.reshape(N_CORES, BP, 1, H, W)
    in_maps = [{"mask_fg": np.ascontiguousarray(shards[i])} for i in range(N_CORES)]
    res = run_bass_kernel_spmd(nc, in_maps, core_ids=list(range(N_CORES)))
    return np.concatenate(
        [res.results[i]["out"] for i in range(N_CORES)], axis=0
    ).astype(np.float32)


# revision 10
# speedup vs baseline: 1.0833x; 1.0351x over previous
"""Trainium2 Bass kernel for nn_BBoxGenerator (segment_reduce).

mask_fg (256, 1, 512, 512) f32 -> boxes (256, 4) f32 [x0, y0, x1, y1].

Sharding: pure data parallel over batch; each of the 8 cores handles 32
images independently, no communication.

Per-core algorithm (32 images, each viewed as SBUF tile (128, 4*512) with
partition p holding rows 4p..4p+3):
  - Threshold+row-count runs split across TWO engines in parallel:
      DVE half:  mask = (m > 0.5) in {0,1} bf16, fused accum_out row sums
      ACT half:  mask = sign(m - 0.5) in {-1,0,1} bf16, fused accum row sums
    (sign-encoded "any" test is sum > -(W-1); identical on data without two
    exact-0.5 pixels sharing a row/col)
  - PE: col sums via 4 matmuls/image with a one-hot (128,32) stationary
    routing image i to PSUM partition row perm(i); DVE images occupy rows
    0..15, ACT images rows 16..31 so thresholds stay contiguous.
  - Finishing (batched): masked min/max of row/col indices, one TensorE
    transpose for the cross-partition row reduction, box expand + empty
    default, final DMA un-permutes rows.
"""

import numpy as np

from concourse import bacc, mybir
from concourse.tile import TileContext
from concourse.bass_utils import run_bass_kernel_spmd

F32 = mybir.dt.float32
BF16 = mybir.dt.bfloat16
I32 = mybir.dt.int32
OP = mybir.AluOpType
AX = mybir.AxisListType
AF = mybir.ActivationFunctionType

N_CORES = 8
B = 256
BP = B // N_CORES  # 32 images per core
H = W = 512
CHUNK = 2  # images per DMA
NCH = BP // CHUNK
IMG_FREE = 4 * W  # 2048 free elems per image (4 rows per partition)
HALF = BP // 2

MIN_BOX = 0.05


def perm_row(i: int) -> int:
    """Result row for image i in the 64-row space (SBUF AP starts must be
    0/32/64/96): DVE images -> rows 0..15, ACT images -> rows 32..47."""
    c, ii = divmod(i, CHUNK)
    return c if ii == 0 else 32 + c


def build_nc():
    nc = bacc.Bacc("TRN2", target_bir_lowering=False, debug=False, num_devices=N_CORES)
    x = nc.declare_dram_parameter("mask_fg", [BP, 1, H, W], F32, isOutput=False)
    out = nc.declare_dram_parameter("out", [BP, 4], F32, isOutput=True)

    # (128, BP, 4, 512): partition p holds rows 4p..4p+3 of each image
    xv = x.ap().rearrange("b one (p a) w -> p (b one) a w", p=128)
    # (16, 2, 4): chunk, image-in-chunk, coord -- for the un-permuting DMA
    outv = out.ap().rearrange("(c k) f -> c k f", k=CHUNK)

    with TileContext(nc) as tc:
        with (
            tc.tile_pool(name="consts", bufs=1) as consts,
            tc.tile_pool(name="imgs", bufs=8) as imgs,
            tc.tile_pool(name="masks", bufs=4) as masks,
            tc.tile_pool(name="small", bufs=1) as small,
            tc.tile_pool(name="pcol", bufs=1, space="PSUM") as pcol_pool,
            tc.tile_pool(name="ptr", bufs=1, space="PSUM") as ptr_pool,
        ):
            # ---- constants ----
            neg_half = consts.tile([128, 1], F32)
            nc.gpsimd.memset(neg_half[:], -0.5)

            hm512_i = consts.tile([128, 256], I32)
            nc.gpsimd.iota(hm512_i[:], [[0, 64], [1, 4]], base=-512, channel_multiplier=4)
            hm512 = consts.tile([128, 256], F32)
            nc.vector.tensor_copy(hm512[:], hm512_i[:])

            hp1_i = consts.tile([128, 256], I32)
            nc.gpsimd.iota(hp1_i[:], [[0, 64], [1, 4]], base=1, channel_multiplier=4)
            hp1 = consts.tile([128, 256], F32)
            nc.vector.tensor_copy(hp1[:], hp1_i[:])

            wm512_i = consts.tile([64, W], I32)
            nc.gpsimd.iota(wm512_i[:], [[1, W]], base=-512, channel_multiplier=0)
            wm512 = consts.tile([64, W], F32)
            nc.vector.tensor_copy(wm512[:], wm512_i[:])

            wp1_i = consts.tile([64, W], I32)
            nc.gpsimd.iota(wp1_i[:], [[1, W]], base=1, channel_multiplier=0)
            wp1 = consts.tile([64, W], F32)
            nc.vector.tensor_copy(wp1[:], wp1_i[:])

            ones128 = consts.tile([128, 128], F32)
            nc.gpsimd.memset(ones128[:], 1.0)
            ident = consts.tile([128, 128], F32)
            nc.gpsimd.affine_select(
                ident[:], ones128[:], [[-1, 128]], OP.is_equal, 0.0,
                base=0, channel_multiplier=1,
            )

            # one-hot stationaries: OH[:, i*64 + perm_row(i)] = 1, else 0
            oh = consts.tile([128, BP * 64], BF16)
            nc.gpsimd.memset(oh[:], 0.0)
            for i in range(BP):
                j = i * 64 + perm_row(i)
                nc.gpsimd.memset(oh[:, j:j + 1], 1.0)

            row_cnt = small.tile([128, 64 * 4], F32)
            psum_col = pcol_pool.tile([64, W], F32)

            # ---- main loop over image chunks ----
            for c in range(NCH):
                img = imgs.tile([128, CHUNK * IMG_FREE], F32)
                nc.sync.dma_start(
                    out=img[:].rearrange("p (b a w) -> p b a w", b=CHUNK, a=4),
                    in_=xv[:, c * CHUNK:(c + 1) * CHUNK],
                )
                m01 = masks.tile([128, CHUNK * IMG_FREE], BF16)
                for ii in range(CHUNK):
                    i = c * CHUNK + ii
                    pr = perm_row(i)
                    for r in range(4):
                        sl = slice(ii * IMG_FREE + r * W, ii * IMG_FREE + (r + 1) * W)
                        acc = row_cnt[:, pr * 4 + r:pr * 4 + r + 1]
                        if ii == 0:
                            nc.vector.tensor_scalar(
                                m01[:, sl], img[:, sl], 0.5, None, OP.is_gt, OP.add,
                                accum_out=acc,
                            )
                        else:
                            nc.scalar.activation(
                                m01[:, sl], img[:, sl], AF.Sign, bias=neg_half[:],
                                accum_out=acc,
                            )
                    for r in range(4):
                        sl = slice(ii * IMG_FREE + r * W, ii * IMG_FREE + (r + 1) * W)
                        nc.tensor.matmul(
                            psum_col[:, :], oh[:, i * 64:(i + 1) * 64], m01[:, sl],
                            start=(i == 0 and r == 0), stop=(i == BP - 1 and r == 3),
                        )

            # ---- finishing ----
            # "any" thresholds: {0,1} rows 0..15 -> cnt > 0.5; sign rows
            # 32..47 -> sum > -511. Unused rows stay zero (memset).
            rtmp = small.tile([128, 256], F32)
            nc.gpsimd.memset(rtmp[:], 0.0)
            rvals = small.tile([128, 128], F32)
            nc.gpsimd.memset(rvals[:], 0.0)
            for lo_col, thr in ((0, 0.5), (128, -511.0)):
                cs = slice(lo_col, lo_col + 64)
                nc.vector.scalar_tensor_tensor(
                    rtmp[:, cs], row_cnt[:, cs], thr, hm512[:, cs], OP.is_gt, OP.mult)
            nc.vector.tensor_reduce(
                rvals[:, 0:64], rtmp[:].rearrange("p (i r) -> p i r", r=4),
                op=OP.min, axis=AX.X)
            for lo_col, thr in ((0, 0.5), (128, -511.0)):
                cs = slice(lo_col, lo_col + 64)
                nc.vector.scalar_tensor_tensor(
                    rtmp[:, cs], row_cnt[:, cs], thr, hp1[:, cs], OP.is_gt, OP.mult)
            nc.vector.tensor_reduce(
                rvals[:, 64:128], rtmp[:].rearrange("p (i r) -> p i r", r=4),
                op=OP.max, axis=AX.X)

            rT = ptr_pool.tile([128, 128], F32)
            nc.tensor.transpose(rT[:], rvals[:], ident[:])

            y_min_v = small.tile([64, 1], F32)
            y_max_v = small.tile([64, 1], F32)
            nc.vector.tensor_reduce(y_min_v[:], rT[0:64, :], op=OP.min, axis=AX.X)
            nc.vector.tensor_reduce(y_max_v[:], rT[64:128, :], op=OP.max, axis=AX.X)

            # col side straight off PSUM sums
            ctmp = small.tile([64, W], F32)
            nc.gpsimd.memset(ctmp[:], 0.0)
            x_min_v = small.tile([64, 1], F32)
            x_max_v = small.tile([64, 1], F32)
            for lo_row, thr in ((0, 0.5), (32, -511.0)):
                ps = slice(lo_row, lo_row + 16)
                nc.vector.scalar_tensor_tensor(
                    ctmp[ps, :], psum_col[ps, :], thr, wm512[ps, :], OP.is_gt, OP.mult)
            nc.vector.tensor_reduce(x_min_v[:], ctmp[:], op=OP.min, axis=AX.X)
            for lo_row, thr in ((0, 0.5), (32, -511.0)):
                ps = slice(lo_row, lo_row + 16)
                nc.vector.scalar_tensor_tensor(
                    ctmp[ps, :], psum_col[ps, :], thr, wp1[ps, :], OP.is_gt, OP.mult)
            nc.vector.tensor_reduce(x_max_v[:], ctmp[:], op=OP.max, axis=AX.X)

            # empty mask (no foreground at all): y_max_v == 0
            emp = small.tile([64, 1], F32)
            nc.vector.tensor_scalar(emp[:], y_max_v[:], 0.5, None, OP.is_lt)

            # normalize to [0,1]: lo = (v + 512)/512, hi = (v - 1)/512
            boxes = small.tile([64, 4], F32)
            nc.vector.tensor_scalar(
                boxes[:, 0:1], x_min_v[:], 512.0, 1.0 / 512, OP.add, OP.mult)
            nc.vector.tensor_scalar(
                boxes[:, 1:2], y_min_v[:], 512.0, 1.0 / 512, OP.add, OP.mult)
            nc.vector.tensor_scalar(
                boxes[:, 2:3], x_max_v[:], 1.0, 1.0 / 512, OP.subtract, OP.mult)
            nc.vector.tensor_scalar(
                boxes[:, 3:4], y_max_v[:], 1.0, 1.0 / 512, OP.subtract, OP.mult)

            # expand too-small boxes per axis
            size_t = small.tile([64, 1], F32)
            too_t = small.tile([64, 1], F32)
            csum_t = small.tile([64, 1], F32)
            lo2_t = small.tile([64, 1], F32)
            hi2_t = small.tile([64, 1], F32)
            d_t = small.tile([64, 1], F32)
            for lo_c, hi_c in ((0, 2), (1, 3)):
                lo = boxes[:, lo_c:lo_c + 1]
                hi = boxes[:, hi_c:hi_c + 1]
                nc.vector.tensor_sub(size_t[:], hi, lo)
                nc.vector.tensor_scalar(too_t[:], size_t[:], MIN_BOX, None, OP.is_lt)
                nc.vector.tensor_add(csum_t[:], lo, hi)
                nc.vector.tensor_scalar(
                    lo2_t[:], csum_t[:], 0.5, MIN_BOX * 0.5, OP.mult, OP.subtract)
                nc.vector.tensor_scalar(lo2_t[:], lo2_t[:], 0.0, None, OP.max)
                nc.vector.tensor_scalar(
                    hi2_t[:], csum_t[:], 0.5, MIN_BOX * 0.5, OP.mult, OP.add)
                nc.vector.tensor_scalar(hi2_t[:], hi2_t[:], 1.0, None, OP.min)
                nc.vector.tensor_sub(d_t[:], lo2_t[:], lo)
                nc.vector.scalar_tensor_tensor(
                    lo, d_t[:], too_t[:], lo, OP.mult, OP.add)
                nc.vector.tensor_sub(d_t[:], hi2_t[:], hi)
                nc.vector.scalar_tensor_tensor(
                    hi, d_t[:], too_t[:], hi, OP.mult, OP.add)

            # default box where empty: final = (default - boxes) * emp + boxes
            dflt = small.tile([64, 4], F32)
            nc.gpsimd.memset(dflt[:, 0:2], 0.25)
            nc.gpsimd.memset(dflt[:, 2:4], 0.75)
            dmb = small.tile([64, 4], F32)
            nc.vector.tensor_sub(dmb[:], dflt[:], boxes[:])
            final = small.tile([64, 4], F32)
            nc.vector.scalar_tensor_tensor(
                final[:], dmb[:], emp[:], boxes[:], OP.mult, OP.add)

            # un-permute: rows 0..15 are (c, ii in 0..1), rows 16..31 (c, ii in 2..3)
            nc.sync.dma_start(out=outv[:, 0:1], in_=final[0:16, :])
            nc.sync.dma_start(out=outv[:, 1:2], in_=final[32:48, :])

    return nc


_NC = None


def _get_nc():
    global _NC
    if _NC is None:
        nc = build_nc()
        nc.compile()
        _NC = nc
    return _NC


def kernel(mask_fg: np.ndarray) -> np.ndarray:
    mask_fg = np.ascontiguousarray(np.asarray(mask_fg, dtype=np.float32))
    assert mask_fg.shape == (B, 1, H, W), mask_fg.shape
    nc = _get_nc()
    shards = mask_fg.reshape(N_CORES, BP, 1, H, W)
    in_maps = [{"mask_fg": np.ascontiguousarray(shards[i])} for i in range(N_CORES)]
    res = run_bass_kernel_spmd(nc, in_maps, core_ids=list(range(N_CORES)))
    return np.concatenate(
        [res.results[i]["out"] for i in range(N_CORES)], axis=0
    ).astype(np.float32)


# revision 11
# speedup vs baseline: 1.0849x; 1.0016x over previous
"""Trainium2 Bass kernel for nn_BBoxGenerator (segment_reduce).

mask_fg (256, 1, 512, 512) f32 -> boxes (256, 4) f32 [x0, y0, x1, y1].

Sharding: pure data parallel over batch; each of the 8 cores handles 32
images independently, no communication.

Per-core algorithm (32 images, each viewed as SBUF tile (128, 4*512) with
partition p holding rows 4p..4p+3):
  - Threshold+row-count runs split across TWO engines in parallel:
      DVE half:  mask = (m > 0.5) in {0,1} bf16, fused accum_out row sums
      ACT half:  mask = sign(m - 0.5) in {-1,0,1} bf16, fused accum row sums
    (sign-encoded "any" test is sum > -(W-1); identical on data without two
    exact-0.5 pixels sharing a row/col)
  - PE: col sums via 4 matmuls/image with a one-hot (128,32) stationary
    routing image i to PSUM partition row perm(i); DVE images occupy rows
    0..15, ACT images rows 16..31 so thresholds stay contiguous.
  - Finishing (batched): masked min/max of row/col indices, one TensorE
    transpose for the cross-partition row reduction, box expand + empty
    default, final DMA un-permutes rows.
"""

import numpy as np

from concourse import bacc, mybir
from concourse.tile import TileContext
from concourse.bass_utils import run_bass_kernel_spmd

F32 = mybir.dt.float32
BF16 = mybir.dt.bfloat16
I32 = mybir.dt.int32
OP = mybir.AluOpType
AX = mybir.AxisListType
AF = mybir.ActivationFunctionType

N_CORES = 8
B = 256
BP = B // N_CORES  # 32 images per core
H = W = 512
CHUNK = 2  # images per DMA
NCH = BP // CHUNK
IMG_FREE = 4 * W  # 2048 free elems per image (4 rows per partition)
HALF = BP // 2

MIN_BOX = 0.05


def perm_row(i: int) -> int:
    """Result row for image i in the 64-row space (SBUF AP starts must be
    0/32/64/96): DVE images -> rows 0..15, ACT images -> rows 32..47."""
    c, ii = divmod(i, CHUNK)
    return c if ii == 0 else 32 + c


def build_nc():
    nc = bacc.Bacc("TRN2", target_bir_lowering=False, debug=False, num_devices=N_CORES)
    x = nc.declare_dram_parameter("mask_fg", [BP, 1, H, W], F32, isOutput=False)
    out = nc.declare_dram_parameter("out", [BP, 4], F32, isOutput=True)

    # (128, BP, 4, 512): partition p holds rows 4p..4p+3 of each image
    xv = x.ap().rearrange("b one (p a) w -> p (b one) a w", p=128)
    # (16, 2, 4): chunk, image-in-chunk, coord -- for the un-permuting DMA
    outv = out.ap().rearrange("(c k) f -> c k f", k=CHUNK)

    with TileContext(nc) as tc:
        with (
            tc.tile_pool(name="consts", bufs=1) as consts,
            tc.tile_pool(name="imgs", bufs=8) as imgs,
            tc.tile_pool(name="masks", bufs=4) as masks,
            tc.tile_pool(name="small", bufs=1) as small,
            tc.tile_pool(name="pcol", bufs=1, space="PSUM") as pcol_pool,
            tc.tile_pool(name="ptr", bufs=1, space="PSUM") as ptr_pool,
        ):
            # ---- constants ----
            neg_half = consts.tile([128, 1], F32)
            nc.gpsimd.memset(neg_half[:], -0.5)

            hm512_i = consts.tile([128, 256], I32)
            nc.gpsimd.iota(hm512_i[:], [[0, 64], [1, 4]], base=-512, channel_multiplier=4)
            hm512 = consts.tile([128, 256], F32)
            nc.vector.tensor_copy(hm512[:], hm512_i[:])

            hp1_i = consts.tile([128, 256], I32)
            nc.gpsimd.iota(hp1_i[:], [[0, 64], [1, 4]], base=1, channel_multiplier=4)
            hp1 = consts.tile([128, 256], F32)
            nc.vector.tensor_copy(hp1[:], hp1_i[:])

            wm512_i = consts.tile([64, W], I32)
            nc.gpsimd.iota(wm512_i[:], [[1, W]], base=-512, channel_multiplier=0)
            wm512 = consts.tile([64, W], F32)
            nc.vector.tensor_copy(wm512[:], wm512_i[:])

            wp1_i = consts.tile([64, W], I32)
            nc.gpsimd.iota(wp1_i[:], [[1, W]], base=1, channel_multiplier=0)
            wp1 = consts.tile([64, W], F32)
            nc.vector.tensor_copy(wp1[:], wp1_i[:])

            ones128 = consts.tile([128, 128], F32)
            nc.gpsimd.memset(ones128[:], 1.0)
            ident = consts.tile([128, 128], F32)
            nc.gpsimd.affine_select(
                ident[:], ones128[:], [[-1, 128]], OP.is_equal, 0.0,
                base=0, channel_multiplier=1,
            )

            # one-hot stationaries: OH[:, i*64 + perm_row(i)] = 1, else 0
            oh = consts.tile([128, BP * 64], BF16)
            nc.gpsimd.memset(oh[:], 0.0)
            for i in range(BP):
                j = i * 64 + perm_row(i)
                nc.gpsimd.memset(oh[:, j:j + 1], 1.0)

            rc_dve = small.tile([128, 64], F32)
            rc_act = small.tile([128, 64], F32)
            psum_col = pcol_pool.tile([64, W], F32)

            # ---- main loop over image chunks ----
            for c in range(NCH):
                img = imgs.tile([128, CHUNK * IMG_FREE], F32)
                nc.sync.dma_start(
                    out=img[:].rearrange("p (b a w) -> p b a w", b=CHUNK, a=4),
                    in_=xv[:, c * CHUNK:(c + 1) * CHUNK],
                )
                m01d = masks.tile([128, IMG_FREE], BF16, tag="m01d")
                m01a = masks.tile([128, IMG_FREE], BF16, tag="m01a")
                for ii in range(CHUNK):
                    i = c * CHUNK + ii
                    m01 = m01d if ii == 0 else m01a
                    rc = rc_dve if ii == 0 else rc_act
                    for r in range(4):
                        src_sl = slice(ii * IMG_FREE + r * W, ii * IMG_FREE + (r + 1) * W)
                        dst_sl = slice(r * W, (r + 1) * W)
                        acc = rc[:, c * 4 + r:c * 4 + r + 1]
                        if ii == 0:
                            nc.vector.tensor_scalar(
                                m01[:, dst_sl], img[:, src_sl], 0.5, None,
                                OP.is_gt, OP.add, accum_out=acc,
                            )
                        else:
                            nc.scalar.activation(
                                m01[:, dst_sl], img[:, src_sl], AF.Sign,
                                bias=neg_half[:], accum_out=acc,
                            )
                    for r in range(4):
                        dst_sl = slice(r * W, (r + 1) * W)
                        nc.tensor.matmul(
                            psum_col[:, :], oh[:, i * 64:(i + 1) * 64], m01[:, dst_sl],
                            start=(i == 0 and r == 0), stop=(i == BP - 1 and r == 3),
                        )

            # ---- finishing ----
            # "any" thresholds: {0,1} rows 0..15 -> cnt > 0.5; sign rows
            # 32..47 -> sum > -511. Unused rows stay zero (memset).
            rtmp = small.tile([128, 256], F32)
            nc.gpsimd.memset(rtmp[:], 0.0)
            rvals = small.tile([128, 128], F32)
            nc.gpsimd.memset(rvals[:], 0.0)
            for lo_col, rc, thr in ((0, rc_dve, 0.5), (128, rc_act, -511.0)):
                cs = slice(lo_col, lo_col + 64)
                nc.vector.scalar_tensor_tensor(
                    rtmp[:, cs], rc[:], thr, hm512[:, 0:64], OP.is_gt, OP.mult)
            nc.vector.tensor_reduce(
                rvals[:, 0:64], rtmp[:].rearrange("p (i r) -> p i r", r=4),
                op=OP.min, axis=AX.X)
            for lo_col, rc, thr in ((0, rc_dve, 0.5), (128, rc_act, -511.0)):
                cs = slice(lo_col, lo_col + 64)
                nc.vector.scalar_tensor_tensor(
                    rtmp[:, cs], rc[:], thr, hp1[:, 0:64], OP.is_gt, OP.mult)
            nc.vector.tensor_reduce(
                rvals[:, 64:128], rtmp[:].rearrange("p (i r) -> p i r", r=4),
                op=OP.max, axis=AX.X)

            rT = ptr_pool.tile([128, 128], F32)
            nc.tensor.transpose(rT[:], rvals[:], ident[:])

            y_min_v = small.tile([64, 1], F32)
            y_max_v = small.tile([64, 1], F32)
            nc.vector.tensor_reduce(y_min_v[:], rT[0:64, :], op=OP.min, axis=AX.X)
            nc.vector.tensor_reduce(y_max_v[:], rT[64:128, :], op=OP.max, axis=AX.X)

            # col side straight off PSUM sums
            ctmp = small.tile([64, W], F32)
            nc.gpsimd.memset(ctmp[:], 0.0)
            x_min_v = small.tile([64, 1], F32)
            x_max_v = small.tile([64, 1], F32)
            for lo_row, thr in ((0, 0.5), (32, -511.0)):
                ps = slice(lo_row, lo_row + 16)
                nc.vector.scalar_tensor_tensor(
                    ctmp[ps, :], psum_col[ps, :], thr, wm512[ps, :], OP.is_gt, OP.mult)
            nc.vector.tensor_reduce(x_min_v[:], ctmp[:], op=OP.min, axis=AX.X)
            for lo_row, thr in ((0, 0.5), (32, -511.0)):
                ps = slice(lo_row, lo_row + 16)
                nc.vector.scalar_tensor_tensor(
                    ctmp[ps, :], psum_col[ps, :], thr, wp1[ps, :], OP.is_gt, OP.mult)
            nc.vector.tensor_reduce(x_max_v[:], ctmp[:], op=OP.max, axis=AX.X)

            # empty mask (no foreground at all): y_max_v == 0
            emp = small.tile([64, 1], F32)
            nc.vector.tensor_scalar(emp[:], y_max_v[:], 0.5, None, OP.is_lt)

            # normalize to [0,1]: lo = (v + 512)/512, hi = (v - 1)/512
            boxes = small.tile([64, 4], F32)
            nc.vector.tensor_scalar(
                boxes[:, 0:1], x_min_v[:], 512.0, 1.0 / 512, OP.add, OP.mult)
            nc.vector.tensor_scalar(
                boxes[:, 1:2], y_min_v[:], 512.0, 1.0 / 512, OP.add, OP.mult)
            nc.vector.tensor_scalar(
                boxes[:, 2:3], x_max_v[:], 1.0, 1.0 / 512, OP.subtract, OP.mult)
            nc.vector.tensor_scalar(
                boxes[:, 3:4], y_max_v[:], 1.0, 1.0 / 512, OP.subtract, OP.mult)

            # expand too-small boxes per axis
            size_t = small.tile([64, 1], F32)
            too_t = small.tile([64, 1], F32)
            csum_t = small.tile([64, 1], F32)
            lo2_t = small.tile([64, 1], F32)
            hi2_t = small.tile([64, 1], F32)
            d_t = small.tile([64, 1], F32)
            for lo_c, hi_c in ((0, 2), (1, 3)):
                lo = boxes[:, lo_c:lo_c + 1]
                hi = boxes[:, hi_c:hi_c + 1]
                nc.vector.tensor_sub(size_t[:], hi, lo)
                nc.vector.tensor_scalar(too_t[:], size_t[:], MIN_BOX, None, OP.is_lt)
                nc.vector.tensor_add(csum_t[:], lo, hi)
                nc.vector.tensor_scalar(
                    lo2_t[:], csum_t[:], 0.5, MIN_BOX * 0.5, OP.mult, OP.subtract)
                nc.vector.tensor_scalar(lo2_t[:], lo2_t[:], 0.0, None, OP.max)
                nc.vector.tensor_scalar(
                    hi2_t[:], csum_t[:], 0.5, MIN_BOX * 0.5, OP.mult, OP.add)
                nc.vector.tensor_scalar(hi2_t[:], hi2_t[:], 1.0, None, OP.min)
                nc.vector.tensor_sub(d_t[:], lo2_t[:], lo)
                nc.vector.scalar_tensor_tensor(
                    lo, d_t[:], too_t[:], lo, OP.mult, OP.add)
                nc.vector.tensor_sub(d_t[:], hi2_t[:], hi)
                nc.vector.scalar_tensor_tensor(
                    hi, d_t[:], too_t[:], hi, OP.mult, OP.add)

            # default box where empty: final = (default - boxes) * emp + boxes
            dflt = small.tile([64, 4], F32)
            nc.gpsimd.memset(dflt[:, 0:2], 0.25)
            nc.gpsimd.memset(dflt[:, 2:4], 0.75)
            dmb = small.tile([64, 4], F32)
            nc.vector.tensor_sub(dmb[:], dflt[:], boxes[:])
            final = small.tile([64, 4], F32)
            nc.vector.scalar_tensor_tensor(
                final[:], dmb[:], emp[:], boxes[:], OP.mult, OP.add)

            # un-permute: rows 0..15 are (c, ii in 0..1), rows 16..31 (c, ii in 2..3)
            nc.sync.dma_start(out=outv[:, 0:1], in_=final[0:16, :])
            nc.sync.dma_start(out=outv[:, 1:2], in_=final[32:48, :])

    return nc


_NC = None


def _get_nc():
    global _NC
    if _NC is None:
        nc = build_nc()
        nc.compile()
        _NC = nc
    return _NC


def kernel(mask_fg: np.ndarray) -> np.ndarray:
    mask_fg = np.ascontiguousarray(np.asarray(mask_fg, dtype=np.float32))
    assert mask_fg.shape == (B, 1, H, W), mask_fg.shape
    nc = _get_nc()
    shards = mask_fg.reshape(N_CORES, BP, 1, H, W)
    in_maps = [{"mask_fg": np.ascontiguousarray(shards[i])} for i in range(N_CORES)]
    res = run_bass_kernel_spmd(nc, in_maps, core_ids=list(range(N_CORES)))
    return np.concatenate(
        [res.results[i]["out"] for i in range(N_CORES)], axis=0
    ).astype(np.float32)
